# revision 12
# baseline (speedup 1.0000x reference)
"""Locally-connected convolution (unshared weights) on 8 Trainium2 NeuronCores.

out[b,o,i,j] = sum_{c,u,v} x[b,c,i+u,j+v] * weight[i,j,o,c,u,v]
  x: [64, 64, 32, 32] f32, weight: [28, 28, 128, 64, 5, 5] f32 -> out [64, 128, 28, 28]

"x-chunk-major" flipped layout: the weight stream (20.07 MB/core fp8, used
exactly once) is the hard floor (~60 us at ~350 GB/s HBM->SBUF), so the PE
must ingest weights faster than DMA delivers them.  Weights are the MOVING
matmul operand (2 concurrent col-tiled streams = 2 cols/cycle @ 2.4 GHz);
x chunks are the stationary operand, and the weight stream is reordered so
one stationary load serves up to 256 moving columns:

  For a stripe of 4 (even, odd) position pairs sharing an output row, the
  PSUM accumulator is one bank [128, 4*O] f32: partitions 0-63 = batch of
  the 4 even positions, 64-127 = odd; free = (pair slot, o).  An x chunk
  [128, 64] at (row h, colpair cp) is consumed by two adjacent even
  positions' weight blocks -> ONE matmul, rhs [128, 256], out two adjacent
  PSUM slots.  LDWEIGHTS (64 cols) hides under the 256-col stream.  Even
  and odd positions ride the two column halves of the PE array
  (tile_position (0,0) / (0,64)) so their streams run concurrently.
  Accumulation relies on per-element has_written: only the first matmul
  per (stripe, half) uses start=True (whole-bank clear); later matmuls
  overwrite-on-first-touch / accumulate, which is order-independent.
  The 5 leftover K=64 lone taps per position run as 64x64 quadrant
  matmuls ((0,0) vs (64,64)), two parities concurrent, sharing [128, O]
  weight blocks.  A nosync dependency chain pins tensor-queue program
  order (PE-array state + bank clears are resources the Tile dependency
  tracker doesn't model).

K decomposition per position (1600 = 25 taps x 64 ch) and the x layout are
unchanged from the pairs kernel: x stored once per core, partitions 0-63 =
channels of EVEN input columns, 64-127 ODD; 10 pair chunks + 5 lone taps.

Schedule: weight DMAs carry ~6400 B/partition segments cut at matmul
boundaries, alternating the two HWDGE queues (16 SDMA engines round-robin:
both queues busy => ~350 GB/s); a deep tile pool keeps DMA independent of
PE progress.  One [128, 4*O] f32->f16 cast per stripe; outputs stream per
stripe via gpsimd.  Output layout: [g*64+b, pair, o] for position 2*pair+g.
"""

import numpy as np

B, C, H, W = 64, 64, 32, 32
ROWS = COLS = 28
O, KH, KW = 128, 5, 5
NCORES = 8
PPC = (ROWS * COLS) // NCORES  # 98 positions per core
NPAIR = PPC // 2               # 49 (even, odd) position pairs
XROWS, XW = 8, 36              # sheared x grid: 8 input rows x 36 cols
PAIRS = XW // 2                # 18 column pairs per sheared row
WSCALE = 32.0                  # weights x32 into E3M4 range; x carries /32
SP = 4                         # pairs per stripe (= one PSUM bank)
WLA = 16                       # wtile pool depth (weight segments in flight)
XHALF = 9 * B                  # x row loaded in two 9-pair halves


def _core_geom(k):
    p0 = PPC * k
    return p0 // COLS, p0 % COLS  # r0 (first input/output row), s0 in {0, 14}


def _pos_slot(t):
    """Relative position t in [0,98) -> (di, w2) grid coords shared by all cores."""
    di, jj = t // COLS, t % COLS
    return di, jj + (4 if jj >= 14 else 0)


def _pair_geom(p):
    """Pair p -> (di, c): positions 2p (w2=2c) and 2p+1 (w2=2c+1), same di."""
    di, w2 = _pos_slot(2 * p)
    return di, w2 // 2


def _stripes():
    """Stripes never straddle output rows (rows are 14 or 7 pairs)."""
    out = []
    for r0 in range(0, NPAIR, 14):
        row = list(range(r0, min(r0 + 14, NPAIR)))
        sizes = [4, 4, 3, 3] if len(row) == 14 else [4, 3]
        i = 0
        for sz in sizes:
            out.append(row[i:i + sz])
            i += sz
    return out


def _stripe_plan(pairs):
    """-> (di, cs, mms, ncols).  mms in emission order; weight columns are
    assigned in the same order (the A/B lone matmuls share one block).

    mm: dict(kind, half, u, cp, slot0, nblk, woff, blocks)
      pair: lhsT = XT[di+u][:, cp*B:(cp+1)*B], rhs = wt[:, woff:woff+nblk*O],
            out = stile[half*64:+64, slot0*O:(slot0+nblk)*O]
      lone: all three operands on partitions [half*64, half*64+64)
    """
    di, _ = _pair_geom(pairs[0])
    cs = [_pair_geom(p)[1] for p in pairs]
    n = len(pairs)
    mms = []
    off = 0
    for u in range(KH):
        per_half = []
        for half in range(2):
            blocks = [(s, q) for s in range(n) for q in range(2)]
            groups, i = [], 0
            while i < len(blocks):
                s, q = blocks[i]
                if (q == 1 and i + 1 < len(blocks)
                        and cs[blocks[i + 1][0]] == cs[s] + 1):
                    groups.append([blocks[i], blocks[i + 1]])
                    i += 2
                else:
                    groups.append([blocks[i]])
                    i += 1
            per_half.append(groups)
        ga, gb = per_half
        for j in range(max(len(ga), len(gb))):
            for half, gl in ((0, ga), (1, gb)):
                if j < len(gl):
                    grp = gl[j]
                    s0, q0 = grp[0]
                    mms.append(dict(kind="pair", half=half, u=u,
                                    cp=cs[s0] + q0 + half, slot0=s0,
                                    nblk=len(grp), woff=off, blocks=grp))
                    off += len(grp) * O
    for u in range(KH):
        for s in range(n):
            for half in range(2):
                mms.append(dict(kind="lone", half=half, u=u,
                                cp=cs[s] + (2 if half == 0 else 0),
                                slot0=s, nblk=1, woff=off, blocks=[(s, None)]))
            off += O  # one [128, O] block shared by the A and B lone matmuls
    return di, cs, mms, off


def _stripe_bases():
    bases, tot = [], 0
    for pairs in _stripes():
        bases.append(tot)
        tot += _stripe_plan(pairs)[3]
    return bases, tot


def _build_xs(x_chwb, k):
    """x_chwb: [C,H,W,B] f32 -> sheared per-core grid [C, XROWS, XW, B]."""
    r0, s0 = _core_geom(k)
    xs = np.zeros((C, XROWS, XW, B), dtype=x_chwb.dtype)
    for h in range(XROWS):
        if s0 == 0:
            xs[:, h, 0:18] = x_chwb[:, r0 + h, 0:18]
            xs[:, h, 18:36] = x_chwb[:, r0 + h, 14:32]
        else:
            xs[:, h, 0:18] = x_chwb[:, r0 + h, 14:32]
            if r0 + h + 1 < H:
                xs[:, h, 18:36] = x_chwb[:, r0 + h + 1, 0:18]
    return xs


def _build_xp(x_chwb, k):
    """-> [XROWS, 128, PAIRS*B] f16, partition g*64+c = channel c of col 2cp+g."""
    xs = _build_xs(x_chwb, k) * np.float32(1.0 / WSCALE)
    xg = xs.reshape(C, XROWS, PAIRS, 2, B).transpose(3, 0, 1, 2, 4)
    xp = xg.reshape(128, XROWS, PAIRS * B).transpose(1, 0, 2)
    return np.ascontiguousarray(xp).astype(np.float16)


def _abs_pos(k, t):
    p = PPC * k + t
    return p // COLS, p % COLS


def _build_wt(weight, k):
    """weight [ROWS,COLS,O,C,KH,KW] f32 -> per-core [128, WTOT] E3M4 in
    x-chunk-major stream order (exactly the _stripe_plan emission order)."""
    import ml_dtypes

    ii, jj = zip(*[_abs_pos(k, t) for t in range(PPC)])
    wc = weight[list(ii), list(jj)]  # [PPC, O, C, KH, KW] f32
    bases, tot = _stripe_bases()
    WT = np.zeros((128, tot), np.float32)
    for si, pairs in enumerate(_stripes()):
        di, cs, mms, ncols = _stripe_plan(pairs)
        base = bases[si]
        for m in mms:
            u = m["u"]
            if m["kind"] == "pair":
                t0 = m["half"]
                for bi, (s, q) in enumerate(m["blocks"]):
                    t = 2 * pairs[s] + t0
                    col = base + m["woff"] + bi * O
                    for g in range(2):
                        v = 2 * q + g + t0
                        # block[g*64+cch, o] = w[t][o, cch, u, v]
                        WT[g * 64:(g + 1) * 64, col:col + O] = \
                            wc[t, :, :, u, v].T
            elif m["half"] == 0:  # fill the shared lone block once
                s = m["slot0"]
                col = base + m["woff"]
                WT[0:64, col:col + O] = wc[2 * pairs[s], :, :, u, 4].T
                WT[64:128, col:col + O] = wc[2 * pairs[s] + 1, :, :, u, 0].T
    wt = np.ascontiguousarray(WT * np.float32(WSCALE))
    return wt.astype(ml_dtypes.float8_e3m4)


def _emulate_core(xp, wt):
    """Pure-numpy emulation of the device program (mirrors AP arithmetic)."""
    xpf = xp.astype(np.float32)
    wtf = wt.astype(np.float32)
    bases, _ = _stripe_bases()
    out = np.zeros((128, NPAIR, O), np.float32)
    for si, pairs in enumerate(_stripes()):
        di, cs, mms, ncols = _stripe_plan(pairs)
        base = bases[si]
        acc = np.zeros((128, len(pairs) * O), np.float32)
        for m in mms:
            h = di + m["u"]
            w0 = base + m["woff"]
            pl = m["half"] * 64
            if m["kind"] == "pair":
                lhsT = xpf[h][:, m["cp"] * B:(m["cp"] + 1) * B]
                rhs = wtf[:, w0:w0 + m["nblk"] * O]
                acc[pl:pl + 64, m["slot0"] * O:(m["slot0"] + m["nblk"]) * O] += \
                    lhsT.T @ rhs
            else:
                lhsT = xpf[h][pl:pl + 64, m["cp"] * B:(m["cp"] + 1) * B]
                rhs = wtf[pl:pl + 64, w0:w0 + O]
                acc[pl:pl + 64, m["slot0"] * O:(m["slot0"] + 1) * O] += \
                    lhsT.T @ rhs
        for s, p in enumerate(pairs):
            out[:, p, :] = acc[:, s * O:(s + 1) * O]
    return out  # [g*64+b, pair, o]; scale folded via x/32 * w*32


def _assemble(outs):
    """list of 8 per-core [128, NPAIR*O] (flat order) -> [B,O,ROWS,COLS] f32."""
    percore = []
    for o in outs:
        a = np.asarray(o, np.float32).reshape(2, B, NPAIR, O)
        percore.append(a.transpose(2, 0, 1, 3).reshape(PPC, B, O))
    full = np.concatenate(percore, axis=0)           # [784, B, O]
    return np.ascontiguousarray(full.transpose(1, 2, 0)).reshape(B, O, ROWS, COLS)


def _segments():
    """One whole-stripe DMA segment per stripe (1.2-1.6 MB transfers keep
    the SDMA engines at line rate with minimal per-transfer overhead);
    stripe 0 is cut at matmul boundaries into three small leading segments
    for a fast pipeline start.  -> list of (stripe_idx, lo, hi) absolute
    weight columns in stream order."""
    bases, tot = _stripe_bases()
    nstripes = len(_stripes())
    segs = []
    for si, pairs in enumerate(_stripes()):
        di, cs, mms, ncols = _stripe_plan(pairs)
        bounds = sorted({m["woff"] for m in mms} | {ncols})
        if si == 0:
            targets = (2048, 6656)
        elif si == nstripes - 1:
            targets = (3200, 6400)  # small tail segments: PE drains fast
        else:
            targets = (6400,)
        cuts = [0]
        for tgt in targets:
            nxt = min((b for b in bounds if b >= tgt), default=ncols)
            if cuts[-1] < nxt < ncols:
                cuts.append(nxt)
        cuts.append(ncols)
        for lo, hi in zip(cuts, cuts[1:]):
            segs.append((si, bases[si] + lo, bases[si] + hi))
    return segs


_PROG_CACHE = {}


def _build_program():
    if "nc" in _PROG_CACHE:
        return _PROG_CACHE["nc"]
    import concourse.bass as bass
    import concourse.tile as tile
    from concourse import bacc, mybir

    f8, f16, f32 = mybir.dt.float8e3, mybir.dt.float16, mybir.dt.float32
    NOSYNC = mybir.DependencyInfo.NO_SYNC_ONLY
    bases, WTOT = _stripe_bases()
    segs = _segments()
    nc = bacc.Bacc("TRN2", target_bir_lowering=False, debug=False, num_devices=NCORES)
    xp_d = nc.dram_tensor("xp", [XROWS, 128, PAIRS * B], f16, kind="ExternalInput")
    wt_d = nc.dram_tensor("wt", [128, WTOT], f8, kind="ExternalInput")
    out_d = nc.dram_tensor("out", [128, NPAIR * O], f16, kind="ExternalOutput")

    with tile.TileContext(nc) as tc:
        with tc.tile_pool(name="xpool", bufs=1) as xpool, \
             tc.tile_pool(name="wpool", bufs=WLA) as wpool, \
             tc.tile_pool(name="opool", bufs=3) as opool, \
             tc.tile_pool(name="psum", bufs=6, space="PSUM") as ppool:
            xp, wt, outp = xp_d.ap(), wt_d.ap(), out_d.ap()
            XT = [xpool.tile([128, PAIRS * B], f16, name=f"x{h}", tag=f"x{h}")
                  for h in range(XROWS)]
            wtiles = [wpool.tile([128, hi - lo], f8, name=f"w{i}", tag="wt")
                      for i, (si, lo, hi) in enumerate(segs)]
            # map absolute weight column -> (segment idx, local offset)
            weng = [nc.scalar, nc.sync]
            qbytes = [0, 0]  # per-queue bytes/partition, greedily balanced

            def seg_of(col):
                for i, (si, lo, hi) in enumerate(segs):
                    if lo <= col < hi:
                        return i, col - lo
                raise AssertionError(col)

            def q_pick(nbytes):
                qi = 0 if qbytes[0] <= qbytes[1] else 1
                qbytes[qi] += nbytes
                return weng[qi]

            def load_x(h, hf):
                q_pick(2 * XHALF).dma_start(
                    XT[h][:, hf * XHALF:(hf + 1) * XHALF],
                    xp[h, :, hf * XHALF:(hf + 1) * XHALF])

            def load_w(i):
                si, lo, hi = segs[i]
                q_pick(hi - lo).dma_start(wtiles[i][:], wt[:, lo:hi])

            # Emission order == per-queue FIFO order.  Stripe 0 (di=0) needs
            # x rows 0-4 first halves + its leading weight segment; stripe 1
            # already reaches the second halves (cp up to 11), so those come
            # right after stripe 0's weights.
            load_w(0)
            for h in range(5):
                load_x(h, 0)
            load_w(1)
            load_w(2)
            for h in range(5):
                load_x(h, 1)
            load_w(3)
            for h in range(5, XROWS):
                load_x(h, 0)
            load_w(4)
            for h in range(5, XROWS):
                load_x(h, 1)
            for i in range(5, len(segs)):
                load_w(i)  # flow-controlled by wpool depth

            # Tensor-queue program order is load-bearing (whole-bank clear
            # on each half's first matmul must precede the rest): chain all
            # tensor instructions with nosync deps.
            tprev = [None]

            def mm(out_ap, lhsT, rhs, start, stop):
                bi = nc.tensor.matmul(out_ap, lhsT, rhs, start=start, stop=stop)
                if tprev[0] is not None:
                    bi.ins.add_dependency(tprev[0], NOSYNC)
                tprev[0] = bi.ins.name
                return bi

            for si, pairs in enumerate(_stripes()):
                di, cs, mms, ncols = _stripe_plan(pairs)
                base = bases[si]
                n = len(pairs)
                stile = ppool.tile([128, n * O], f32, name="ps", tag="ps")
                otile = opool.tile([128, n * O], f16, name="ot", tag="ot")
                seen = [False, False]
                nlone = [0, 0]
                for m in mms:
                    h = di + m["u"]
                    gi, loc = seg_of(base + m["woff"])
                    wti = wtiles[gi]
                    pl = m["half"] * 64
                    start = not seen[m["half"]]
                    seen[m["half"]] = True
                    if m["kind"] == "pair":
                        mm(stile[pl:pl + 64,
                                 m["slot0"] * O:(m["slot0"] + m["nblk"]) * O],
                           XT[h][:, m["cp"] * B:(m["cp"] + 1) * B],
                           wti[:, loc:loc + m["nblk"] * O],
                           start, False)
                    else:
                        nlone[m["half"]] += 1
                        stop = nlone[m["half"]] == KH * n
                        mm(stile[pl:pl + 64,
                                 m["slot0"] * O:(m["slot0"] + 1) * O],
                           XT[h][pl:pl + 64, m["cp"] * B:(m["cp"] + 1) * B],
                           wti[pl:pl + 64, loc:loc + O],
                           start, stop)
                nc.vector.tensor_copy(otile[:], stile[:])
                oeng = nc.sync if si == len(_stripes()) - 1 else nc.gpsimd
                oeng.dma_start(
                    outp[:, pairs[0] * O:(pairs[0] + n) * O], otile[:])

    nc.compile()
    _PROG_CACHE["nc"] = nc
    return nc


def _make_in_maps(x, weight):
    x_chwb = np.ascontiguousarray(
        np.asarray(x, np.float32).transpose(1, 2, 3, 0))
    w32 = np.asarray(weight, np.float32)
    return [{"xp": _build_xp(x_chwb, k), "wt": _build_wt(w32, k)}
            for k in range(NCORES)]


def kernel(x, weight):
    from concourse.bass_utils import run_bass_kernel_spmd

    nc = _build_program()
    in_maps = _make_in_maps(x, weight)
    res = run_bass_kernel_spmd(nc, in_maps, core_ids=list(range(NCORES)))
    return _assemble([res.results[k]["out"] for k in range(NCORES)])
